# revision 1
# baseline (speedup 1.0000x reference)
"""Trainium2 Bass kernel for nn_CustomSelfAttention (sparse-bias attention).

Sharding (8 cores): 4 head-groups (3 heads each) x 2 query-halves (2048 each).
Each core computes its heads' attention for its query half in S^T layout
(keys on partitions, queries on free dim), with the attention bias added as a
dense bf16 tile during PSUM eviction, exp on ACT with fused 1/8 scale, the
softmax denominator via a ones-column appended to V, and a row-parallel
out-projection partial. Host assembles: sum partials over head groups per
query half, concat halves, add bv@Wo.T + bo.

All matmuls run in float32r (1 cycle/row on TRN2 vs 4 for fp32). Projections
pack [K_h|V_h] into one 128-wide stationary. K/Q/V tiles are split per token
block so attention overlaps the projection phase. bk shifts scores by a
per-query constant (softmax-invariant, dropped exactly); bq enters as an
extra score row beta_j = (Wk_h bq_h) . x_j, emitted only when bq != 0;
bv/bo are exact host-side post-adds.
"""

import numpy as np

# problem shapes (hardcoded per contract)
B, N, E, H, D = 1, 4096, 768, 12, 64
NG, NS = 4, 2           # head-group axis x query-half axis = 8 cores
HG = H // NG            # 3 heads per group
DG = HG * D             # 192
Q = N // NS             # 2048 queries per core
KC = N // 128           # 32 key chunks
SCALE = float(D) ** -0.5

_prog_cache = {}


def _legalize_waits(nc, mybir, max_waits=1):
    """Split multi-wait sync_info into preceding 1-wait NoOps (TRN2 TPB
    instructions encode a single sem-wait slot; this walrus build rejects
    more)."""
    counter = 0
    n_split = 0
    for bb in nc.main_func.blocks:
        out = []
        changed = False
        for inst in bb.instructions:
            si = getattr(inst, "sync_info", None)
            if si is not None and si.on_wait and len(si.on_wait) > max_waits:
                waits = list(si.on_wait)
                for w in waits[:-max_waits]:
                    counter += 1
                    nop = mybir.InstNoOp(
                        name=f"legalize-nop-{id(nc)}-{counter}", ins=[], outs=[]
                    )
                    nop.engine = inst.engine
                    nop.sync_info = mybir.SyncInfo(on_wait=[w], on_update=[])
                    nop.bass_nofuse = True
                    try:
                        nc.register_instruction(nop, overwrite=True)
                    except Exception:
                        pass
                    out.append(nop)
                inst.sync_info = mybir.SyncInfo(
                    on_wait=waits[-max_waits:], on_update=si.on_update
                )
                n_split += 1
                changed = True
            out.append(inst)
        if changed:
            bb.instructions = out
    return n_split


def _build_program(has_bq):
    import concourse.bass as bass
    import concourse.mybir as mybir
    import concourse.tile as tile

    F32 = mybir.dt.float32
    F32R = mybir.dt.float32r
    BF16 = mybir.dt.bfloat16
    EXP = mybir.ActivationFunctionType.Exp

    KR = 65 if has_bq else 64   # score contraction rows (d + optional beta)

    nc = bass.Bass(target_bir_lowering=False, debug=True)

    xT = nc.dram_tensor("xT", [E, N], F32, kind="ExternalInput")
    xTq = nc.dram_tensor("xTq", [E, Q], F32, kind="ExternalInput")
    wkv = nc.dram_tensor("wkv", [E, HG * 128], F32, kind="ExternalInput")
    wq = nc.dram_tensor("wq", [E, DG], F32, kind="ExternalInput")
    wb = nc.dram_tensor("wb", [E, 96], F32, kind="ExternalInput")
    wo = nc.dram_tensor("wo", [DG, E], F32, kind="ExternalInput")
    bt = nc.dram_tensor("bt", [N, Q], BF16, kind="ExternalInput")
    ones_a = nc.dram_tensor("ones_a", [128, KC * HG], F32, kind="ExternalInput")
    ones_b = nc.dram_tensor("ones_b", [1, Q], F32, kind="ExternalInput")
    ident = nc.dram_tensor("ident", [128, 128], F32, kind="ExternalInput")
    outp = nc.dram_tensor("outp", [Q, E], F32, kind="ExternalOutput")

    EC = E // 128  # 6 contraction chunks for projections
    TB = N // 512  # 8 token blocks
    QB = Q // 512  # 4 query blocks

    with tile.TileContext(nc) as tc:
        with tc.tile_pool(name="persist", bufs=1) as persist:
            # --- resident weights/constants ---
            wkv_sb = persist.tile([128, EC, HG * 128], F32R)
            wq_sb = persist.tile([128, EC, DG], F32R)
            wb_sb = persist.tile([128, EC, 96], F32R)
            wo_sb = persist.tile([64, HG, E], F32R)
            id_sb = persist.tile([64, 64], F32R)
            ones_sb = persist.tile([1, 96], F32R)
            nc.sync.dma_start(
                out=wkv_sb, in_=wkv[:, :].bitcast(F32R).rearrange("(c p) n -> p c n", p=128))
            nc.sync.dma_start(
                out=wq_sb, in_=wq[:, :].bitcast(F32R).rearrange("(c p) n -> p c n", p=128))
            nc.sync.dma_start(
                out=wb_sb, in_=wb[:, :].bitcast(F32R).rearrange("(c p) n -> p c n", p=128))
            nc.sync.dma_start(
                out=wo_sb, in_=wo[:, :].bitcast(F32R).rearrange("(h p) n -> p h n", p=64))
            nc.sync.dma_start(out=id_sb, in_=ident[0:64, 0:64].bitcast(F32R))
            nc.sync.dma_start(out=ones_sb, in_=ones_a[0:1, 0:96].bitcast(F32R))

            # K^T / Q^T whole per head (coarse deps keep the projection and
            # attention phases serial — fine-grained splits caused PE
            # head-of-line blocking); V-hat one tile over all key chunks
            kT = [persist.tile([KR, N], F32R, tag=f"kT{h}", name=f"kT{h}")
                  for h in range(HG)]
            qT = [persist.tile([KR, Q], F32R, tag=f"qT{h}", name=f"qT{h}")
                  for h in range(HG)]
            vt = persist.tile([128, KC, HG, 65], F32R)
            nc.sync.dma_start(
                out=vt[:, :, :, 64:65],
                in_=ones_a[:, :].bitcast(F32R)
                .rearrange("p (c h) -> p c h", h=HG)[:, :, :, None])
            if has_bq:
                for h in range(HG):
                    nc.sync.dma_start(out=qT[h][64:65, :],
                                      in_=ones_b[:, :].bitcast(F32R))

            # ---------- projections (packed [K_h | V_h] stationaries) ----------
            with tc.tile_pool(name="pj_kv", bufs=3, space="PSUM") as pj_kv, \
                 tc.tile_pool(name="pj_tr", bufs=2, space="PSUM") as pj_tr, \
                 tc.tile_pool(name="xstream", bufs=2) as xstream, \
                 tc.tile_pool(name="vtmp_pool", bufs=2) as vtmp_pool:
                # Q^T first (attention block b needs only its own q block)
                for tb in range(QB):
                    xs = [xstream.tile([128, 512], F32R, tag=f"xq{ec}", name=f"xq{ec}")
                          for ec in range(EC)]
                    for ec in range(EC):
                        nc.sync.dma_start(
                            out=xs[ec],
                            in_=xTq[:, :].bitcast(F32R)[128 * ec:128 * (ec + 1),
                                                        512 * tb:512 * (tb + 1)])
                    aq = pj_kv.tile([128, 512], F32, tag="acc", name="aq")
                    aq2 = pj_kv.tile([64, 512], F32, tag="acc2", bufs=1, name="aq2")
                    for ec in range(EC):
                        nc.tensor.matmul(aq, wq_sb[:, ec, 0:128], xs[ec],
                                         start=(ec == 0), stop=(ec == EC - 1))
                        nc.tensor.matmul(aq2, wq_sb[:, ec, 128:192], xs[ec],
                                         start=(ec == 0), stop=(ec == EC - 1))
                    nc.scalar.copy(qT[0][0:64, 512 * tb:512 * (tb + 1)], aq[0:64, :])
                    nc.scalar.copy(qT[1][0:64, 512 * tb:512 * (tb + 1)], aq[64:128, :])
                    nc.scalar.copy(qT[2][0:64, 512 * tb:512 * (tb + 1)], aq2)
                # K^T + V per token block
                for tb in range(TB):
                    xs = [xstream.tile([128, 512], F32R, tag=f"xq{ec}", name=f"xs{ec}")
                          for ec in range(EC)]
                    for ec in range(EC):
                        nc.sync.dma_start(
                            out=xs[ec],
                            in_=xT[:, :].bitcast(F32R)[128 * ec:128 * (ec + 1),
                                                       512 * tb:512 * (tb + 1)])
                    akv = [pj_kv.tile([128, 512], F32, tag="acc", name="akv")
                           for _ in range(HG)]
                    ab = pj_kv.tile([96, 512], F32, tag="accb", bufs=1, name="ab") if has_bq else None
                    for ec in range(EC):
                        for h in range(HG):
                            nc.tensor.matmul(
                                akv[h], wkv_sb[:, ec, 128 * h:128 * (h + 1)], xs[ec],
                                start=(ec == 0), stop=(ec == EC - 1))
                        if has_bq:
                            nc.tensor.matmul(ab, wb_sb[:, ec, :], xs[ec],
                                             start=(ec == 0), stop=(ec == EC - 1))
                    for h in range(HG):
                        nc.scalar.copy(
                            kT[h][0:64, 512 * tb:512 * (tb + 1)], akv[h][0:64, :])
                        if has_bq:
                            nc.scalar.copy(
                                kT[h][64:65, 512 * tb:512 * (tb + 1)],
                                ab[32 * h:32 * h + 1, :])
                        vtmp = vtmp_pool.tile([64, 512], F32R, tag="vtmp", name="vtmp")
                        nc.vector.tensor_copy(vtmp, akv[h][64:128, :])
                        # transpose V^T [64,512] into V-nat chunks [128,64] x4,
                        # evicting chunk pairs with one strided copy each
                        for c4 in range(0, 4, 2):
                            ptr = pj_tr.tile([128, 2, 64], F32R, tag="ptr", name="ptr")
                            nc.tensor.transpose(
                                ptr[:, 0, :], vtmp[:, 128 * c4:128 * (c4 + 1)], id_sb)
                            nc.tensor.transpose(
                                ptr[:, 1, :], vtmp[:, 128 * (c4 + 1):128 * (c4 + 2)], id_sb)
                            c = tb * 4 + c4
                            nc.vector.tensor_copy(vt[:, c:c + 2, h, 0:64], ptr)

            # ---------- attention ----------
            with tc.tile_pool(name="ps_main", bufs=2, space="PSUM") as ps_main, \
                 tc.tile_pool(name="ps_oaug", bufs=3, space="PSUM") as ps_oaug, \
                 tc.tile_pool(name="ps_rb", bufs=1, space="PSUM") as ps_rb, \
                 tc.tile_pool(name="bpool", bufs=2) as bpool, \
                 tc.tile_pool(name="spool", bufs=2) as spool, \
                 tc.tile_pool(name="ppool", bufs=2) as ppool, \
                 tc.tile_pool(name="npool", bufs=2) as npool, \
                 tc.tile_pool(name="opool", bufs=2) as opool:
                for b in range(QB):
                    oaug = [ps_oaug.tile([65, 512], F32, tag="oaug", name="oaug")
                            for _ in range(HG)]
                    for g in range(8):  # groups of 4 key chunks
                        btile = bpool.tile([128, 4, 512], BF16, tag="bt", name="btile")
                        nc.sync.dma_start(
                            out=btile,
                            in_=bt[512 * g:512 * (g + 1), 512 * b:512 * (b + 1)]
                            .rearrange("(j p) q -> p j q", p=128))
                        for h in range(HG):
                            st = spool.tile([128, 4, 512], F32, tag="st", name="st")
                            for jj in range(2):
                                ps = ps_main.tile([128, 2, 512], F32, tag="ps", name="ps")
                                for j2 in range(2):
                                    c = 4 * g + 2 * jj + j2
                                    nc.tensor.matmul(
                                        ps[:, j2, :],
                                        kT[h][:, 128 * c:128 * (c + 1)],
                                        qT[h][:, 512 * b:512 * (b + 1)],
                                        start=True, stop=True)
                                nc.vector.tensor_add(
                                    st[:, 2 * jj:2 * jj + 2, :], ps,
                                    btile[:, 2 * jj:2 * jj + 2, :])
                            pt = ppool.tile([128, 4, 512], F32R, tag="pt", name="pt")
                            nc.scalar.activation(pt, st, EXP, scale=SCALE)
                            for j in range(4):
                                c = 4 * g + j
                                nc.tensor.matmul(
                                    oaug[h], vt[:, c, h, :], pt[:, j, :],
                                    start=(c == 0), stop=(c == KC - 1))
                    # normalize each head's output slab: recip of the ones-row,
                    # PE-broadcast it across 64 partitions, multiply
                    otn = []
                    for h in range(HG):
                        rec = npool.tile([1, 512], F32R, tag="rec", bufs=2, name="rec")
                        with nc.allow_low_precision(reason="f32r is f32 bits"):
                            nc.vector.reciprocal(rec, oaug[h][64:65, :])
                        rbp = ps_rb.tile([64, 512], F32, tag="rbp", bufs=1, name="rbp")
                        nc.tensor.matmul(rbp, ones_sb[0:1, 0:64], rec,
                                         start=True, stop=True)
                        recb = npool.tile([64, 512], F32, tag="recb", bufs=2, name="recb")
                        nc.scalar.copy(recb, rbp)
                        on = npool.tile([64, 512], F32R, tag="otn", bufs=4, name="on")
                        nc.vector.tensor_mul(on, oaug[h][0:64, :], recb)
                        otn.append(on)
                    # out-projection, all heads accumulated in PSUM
                    for t in range(4):
                        po = ps_main.tile([128, 768], F32, tag="ps", name="po")
                        for h in range(HG):
                            for e0, e1 in ((0, 512), (512, 768)):
                                nc.tensor.matmul(
                                    po[:, e0:e1],
                                    otn[h][:, 128 * t:128 * (t + 1)],
                                    wo_sb[:, h, e0:e1],
                                    start=(h == 0), stop=(h == HG - 1))
                        osb = opool.tile([128, 768], F32, tag="os", name="osb")
                        nc.scalar.copy(osb, po)
                        qrow = (b * 4 + t) * 128
                        nc.sync.dma_start(out=outp[qrow:qrow + 128, :], in_=osb)

    _legalize_waits(nc, mybir)
    return nc


def _host_prep(inputs):
    import ml_dtypes

    x = np.asarray(inputs["x"], dtype=np.float32)[0]          # [N, E]
    sm = np.asarray(inputs["similarity_matrix"]).astype(np.int64)  # [N, 5, 2]
    Wq = np.asarray(inputs["Wq"], dtype=np.float32)
    bq = np.asarray(inputs["bq"], dtype=np.float32)
    Wk = np.asarray(inputs["Wk"], dtype=np.float32)
    Wv = np.asarray(inputs["Wv"], dtype=np.float32)
    Wo = np.asarray(inputs["Wo"], dtype=np.float32)

    has_bq = True  # 65-row operands stream faster than 64 (SBUF ports)
    xT = np.ascontiguousarray(x.T)                            # [E, N]

    # dense bias matrix, pre-scaled by 1/SCALE so the ACT fused scale
    # recovers it exactly: exp(SCALE*(qk + 8*count)) = exp(SCALE*qk + count)
    idx = sm.reshape(N, -1)
    vals = np.where(idx < N, 1.0, 0.0).astype(np.float32)
    safe = np.minimum(idx, N - 1)
    Bm = np.zeros((N, N), dtype=np.float32)
    np.add.at(Bm, (np.repeat(np.arange(N), idx.shape[1]), safe.reshape(-1)),
              vals.reshape(-1))
    BT = np.ascontiguousarray(Bm.T) * (1.0 / SCALE)           # [keys, queries]

    in_maps = []
    for core in range(8):
        g, s = core // NS, core % NS
        gcols = slice(g * DG, (g + 1) * DG)
        wq_np = np.ascontiguousarray(Wq[gcols, :].T)          # [E, 192]
        wkv_np = np.zeros((E, HG * 128), dtype=np.float32)
        wb_np = np.zeros((E, 96), dtype=np.float32)
        for h in range(HG):
            hc = slice((g * HG + h) * D, (g * HG + h + 1) * D)
            wkv_np[:, 128 * h:128 * h + 64] = Wk[hc, :].T
            wkv_np[:, 128 * h + 64:128 * h + 128] = Wv[hc, :].T
            wb_np[:, 32 * h] = Wk[hc, :].T @ bq[hc]           # beta weights
        wo_np = np.ascontiguousarray(Wo[:, gcols].T)          # [192, E]
        in_maps.append({
            "xT": xT,
            "xTq": np.ascontiguousarray(xT[:, s * Q:(s + 1) * Q]),
            "wkv": wkv_np, "wq": wq_np, "wb": wb_np, "wo": wo_np,
            "bt": np.ascontiguousarray(BT[:, s * Q:(s + 1) * Q]).astype(
                ml_dtypes.bfloat16),
            "ones_a": np.ones((128, KC * HG), dtype=np.float32),
            "ones_b": np.ones((1, Q), dtype=np.float32),
            "ident": np.eye(128, dtype=np.float32),
        })
    return in_maps, has_bq


def kernel(**inputs):
    from concourse.bass_utils import run_bass_kernel_spmd

    in_maps, has_bq = _host_prep(inputs)
    key = ("prog", has_bq)
    if key not in _prog_cache:
        _prog_cache[key] = _build_program(has_bq)
    nc = _prog_cache[key]

    res = run_bass_kernel_spmd(nc, in_maps, list(range(8)))

    bv = np.asarray(inputs["bv"], dtype=np.float32)
    bo = np.asarray(inputs["bo"], dtype=np.float32)
    Wo = np.asarray(inputs["Wo"], dtype=np.float32)

    full = np.zeros((N, E), dtype=np.float32)
    for core in range(8):
        s = core % NS
        full[s * Q:(s + 1) * Q, :] += res.results[core]["outp"]
    full += (bv @ Wo.T + bo)[None, :]
    return full.reshape(B, N, E)



# revision 17
# speedup vs baseline: 1.1068x; 1.1068x over previous
"""Trainium2 Bass kernel for nn_CustomSelfAttention (sparse-bias attention).

Sharding (8 cores): 4 head-groups (3 heads each) x 2 query-halves (2048 each).
Each core computes its heads' attention for its query half in S^T layout
(keys on partitions, queries on free dim).

v2 design vs baseline:
- All matmul operands bf16 (same 1 cycle/row as f32r on the PE, but half the
  LDWEIGHTS bytes and SBUF traffic; f32 PSUM accumulation throughout).
- The sparse attention bias is applied POST-exp as a bf16 multiply on the DVE
  (all-SBUF 2-byte operands hit the DVE 2x_1p fast path) with the host
  shipping exp(bias) instead of bias. This replaces the baseline's PSUM f32
  tensor_add (1.4ns/elem) with a 0.5ns/elem multiply and takes the DVE off
  the PSUM port.
- Softmax denominators for the 3 heads are gathered by SB->SB DMA into one
  tile and inverted with a single reciprocal_approx_fast per query block.
- bq enters as a 65th contraction row (beta_j = (Wk_h bq_h) . x_j) only when
  bq != 0; bk drops exactly (softmax-invariant); bv/bo are host post-adds.
- Normalize/out-proj of block b is emitted AFTER block b+1's first score
  group so the PE never idles at block boundaries (keeps the PE p-state up).
Host assembles: sum partials over head groups per query half, concat halves,
add bv@Wo.T + bo.
"""

import numpy as np

# problem shapes (hardcoded per contract)
B, N, E, H, D = 1, 4096, 768, 12, 64
NG, NS = 4, 2           # head-group axis x query-half axis = 8 cores
HG = H // NG            # 3 heads per group
DG = HG * D             # 192
Q = N // NS             # 2048 queries per core
KC = N // 128           # 32 key chunks
SCALE = float(D) ** -0.5

_prog_cache = {}


def _legalize_waits(nc, mybir, max_waits=1):
    """Split multi-wait sync_info into preceding 1-wait NoOps (TRN2 TPB
    instructions encode a single sem-wait slot; this walrus build rejects
    more)."""
    counter = 0
    n_split = 0
    for bb in nc.main_func.blocks:
        out = []
        changed = False
        for inst in bb.instructions:
            si = getattr(inst, "sync_info", None)
            if si is not None and si.on_wait and len(si.on_wait) > max_waits:
                waits = list(si.on_wait)
                for w in waits[:-max_waits]:
                    counter += 1
                    nop = mybir.InstNoOp(
                        name=f"legalize-nop-{id(nc)}-{counter}", ins=[], outs=[]
                    )
                    nop.engine = inst.engine
                    nop.sync_info = mybir.SyncInfo(on_wait=[w], on_update=[])
                    nop.bass_nofuse = True
                    try:
                        nc.register_instruction(nop, overwrite=True)
                    except Exception:
                        pass
                    out.append(nop)
                inst.sync_info = mybir.SyncInfo(
                    on_wait=waits[-max_waits:], on_update=si.on_update
                )
                n_split += 1
                changed = True
            out.append(inst)
        if changed:
            bb.instructions = out
    return n_split


def _build_program(has_bq):
    import concourse.bass as bass
    import concourse.mybir as mybir
    import concourse.tile as tile

    F32 = mybir.dt.float32
    BF16 = mybir.dt.bfloat16
    EXP = mybir.ActivationFunctionType.Exp
    LN = mybir.ActivationFunctionType.Ln

    KR = 65 if has_bq else 64   # score contraction rows (d + optional beta)

    nc = bass.Bass(target_bir_lowering=False, debug=True)

    xT = nc.dram_tensor("xT", [E, N], BF16, kind="ExternalInput")
    xTq = nc.dram_tensor("xTq", [E, Q], BF16, kind="ExternalInput")
    wkv = nc.dram_tensor("wkv", [E, HG * 128], BF16, kind="ExternalInput")
    wq = nc.dram_tensor("wq", [E, DG], BF16, kind="ExternalInput")
    wb = nc.dram_tensor("wb", [E, 96], BF16, kind="ExternalInput")
    wo = nc.dram_tensor("wo", [DG, E], BF16, kind="ExternalInput")
    bt = nc.dram_tensor("bt", [N, Q], BF16, kind="ExternalInput")
    ones_a = nc.dram_tensor("ones_a", [128, KC * HG], BF16, kind="ExternalInput")
    ones_f = nc.dram_tensor("ones_f", [1, 64], F32, kind="ExternalInput")
    ones_b = nc.dram_tensor("ones_b", [3, Q], BF16, kind="ExternalInput")
    ident = nc.dram_tensor("ident", [64, 64], BF16, kind="ExternalInput")
    outp = nc.dram_tensor("outp", [Q, E], F32, kind="ExternalOutput")

    EC = E // 128  # 6 contraction chunks for projections
    TB = N // 512  # 8 token blocks
    QB = Q // 512  # 4 query blocks

    with tile.TileContext(nc) as tc:
        with tc.tile_pool(name="persist", bufs=1) as persist:
            # --- resident weights/constants (all bf16) ---
            wkv_sb = persist.tile([128, EC, HG * 128], BF16)
            wq_sb = persist.tile([128, EC, DG], BF16)
            wb_sb = persist.tile([128, EC, 96], BF16)
            wo_sb = persist.tile([64, HG, E], BF16)
            id_sb = persist.tile([64, 64], BF16)
            ones_sb = persist.tile([1, 64], mybir.dt.float32r)
            nc.sync.dma_start(out=ones_sb, in_=ones_f[:, :].bitcast(mybir.dt.float32r))
            nc.sync.dma_start(
                out=wkv_sb, in_=wkv[:, :].rearrange("(c p) n -> p c n", p=128))
            nc.sync.dma_start(
                out=wq_sb, in_=wq[:, :].rearrange("(c p) n -> p c n", p=128))
            nc.sync.dma_start(
                out=wb_sb, in_=wb[:, :].rearrange("(c p) n -> p c n", p=128))
            nc.sync.dma_start(
                out=wo_sb, in_=wo[:, :].rearrange("(h p) n -> p h n", p=64))
            nc.sync.dma_start(out=id_sb, in_=ident[:, :])

            # K^T / Q^T whole per head; V-hat one tile over all key chunks
            kT = [persist.tile([KR, N], BF16, tag=f"kT{h}", name=f"kT{h}")
                  for h in range(HG)]
            qT = [persist.tile([KR, Q], BF16, tag=f"qT{h}", name=f"qT{h}")
                  for h in range(HG)]
            vt = persist.tile([128, KC, HG, 65], BF16)
            nc.sync.dma_start(
                out=vt[:, :, :, 64:65],
                in_=ones_a[:, :].rearrange("p (c h) -> p c h", h=HG)[:, :, :, None])
            if has_bq:
                for h in range(HG):
                    nc.sync.dma_start(out=qT[h][64:65, :],
                                      in_=ones_b[0:1, :])

            # ---------- projections (packed [K_h | V_h] stationaries) ----------
            with tc.tile_pool(name="pj_kv", bufs=3, space="PSUM") as pj_kv, \
                 tc.tile_pool(name="pj_tr", bufs=2, space="PSUM") as pj_tr, \
                 tc.tile_pool(name="xstream", bufs=2) as xstream, \
                 tc.tile_pool(name="vtmp_pool", bufs=2) as vtmp_pool:
                # Q^T first (attention block b needs only its own q block)
                for tb in range(QB):
                    xs = [xstream.tile([128, 512], BF16, tag=f"xq{ec}", name=f"xq{ec}")
                          for ec in range(EC)]
                    for ec in range(EC):
                        nc.sync.dma_start(
                            out=xs[ec],
                            in_=xTq[128 * ec:128 * (ec + 1),
                                    512 * tb:512 * (tb + 1)])
                    aq = pj_kv.tile([128, 512], F32, tag="acc", name="aq")
                    aq2 = pj_kv.tile([64, 512], F32, tag="acc2", bufs=1, name="aq2")
                    for ec in range(EC):
                        nc.tensor.matmul(aq, wq_sb[:, ec, 0:128], xs[ec],
                                         start=(ec == 0), stop=(ec == EC - 1))
                        nc.tensor.matmul(aq2, wq_sb[:, ec, 128:192], xs[ec],
                                         start=(ec == 0), stop=(ec == EC - 1))
                    nc.scalar.copy(qT[0][0:64, 512 * tb:512 * (tb + 1)], aq[0:64, :])
                    nc.scalar.copy(qT[1][0:64, 512 * tb:512 * (tb + 1)], aq[64:128, :])
                    nc.scalar.copy(qT[2][0:64, 512 * tb:512 * (tb + 1)], aq2)
                # K^T + V per token block
                for tb in range(TB):
                    xs = [xstream.tile([128, 512], BF16, tag=f"xq{ec}", name=f"xs{ec}")
                          for ec in range(EC)]
                    for ec in range(EC):
                        nc.sync.dma_start(
                            out=xs[ec],
                            in_=xT[128 * ec:128 * (ec + 1),
                                   512 * tb:512 * (tb + 1)])
                    akv = [pj_kv.tile([128, 512], F32, tag="acc", name="akv")
                           for _ in range(HG)]
                    ab = pj_kv.tile([96, 512], F32, tag="accb", bufs=1, name="ab") if has_bq else None
                    for ec in range(EC):
                        for h in range(HG):
                            nc.tensor.matmul(
                                akv[h], wkv_sb[:, ec, 128 * h:128 * (h + 1)], xs[ec],
                                start=(ec == 0), stop=(ec == EC - 1))
                        if has_bq:
                            nc.tensor.matmul(ab, wb_sb[:, ec, :], xs[ec],
                                             start=(ec == 0), stop=(ec == EC - 1))
                    for h in range(HG):
                        nc.scalar.copy(
                            kT[h][0:64, 512 * tb:512 * (tb + 1)], akv[h][0:64, :])
                        if has_bq:
                            nc.scalar.copy(
                                kT[h][64:65, 512 * tb:512 * (tb + 1)],
                                ab[32 * h:32 * h + 1, :])
                        vtmp = vtmp_pool.tile([64, 512], BF16, tag="vtmp", name="vtmp")
                        nc.vector.tensor_copy(vtmp, akv[h][64:128, :])
                        # transpose V^T [64,512] into V-nat chunks [128,64] x4,
                        # evicting chunk pairs with one strided copy each
                        for c4 in range(0, 4, 2):
                            ptr = pj_tr.tile([128, 2, 64], BF16, tag="ptr", name="ptr")
                            nc.tensor.transpose(
                                ptr[:, 0, :], vtmp[:, 128 * c4:128 * (c4 + 1)], id_sb)
                            nc.tensor.transpose(
                                ptr[:, 1, :], vtmp[:, 128 * (c4 + 1):128 * (c4 + 2)], id_sb)
                            c = tb * 4 + c4
                            nc.vector.tensor_copy(vt[:, c:c + 2, h, 0:64], ptr)

            # ---------- attention ----------
            with tc.tile_pool(name="ps_main", bufs=2, space="PSUM") as ps_main, \
                 tc.tile_pool(name="ps_oaug", bufs=3, space="PSUM") as ps_oaug, \
                 tc.tile_pool(name="ps_rb", bufs=1, space="PSUM") as ps_rb, \
                 tc.tile_pool(name="bpool", bufs=3) as bpool, \
                 tc.tile_pool(name="spool", bufs=2) as spool, \
                 tc.tile_pool(name="ppool", bufs=2) as ppool, \
                 tc.tile_pool(name="npool", bufs=2) as npool, \
                 tc.tile_pool(name="opool", bufs=2) as opool:

                oaug = {}   # keyed by qblock b; rotation via pool bufs

                def emit_score_group(b, g):
                    """scores + exp + bias-multiply for (qblock b, key group g).
                    Returns the 3 heads' pt tiles (bf16 probs, [128, 4, 512])."""
                    btile = bpool.tile([128, 4, 512], BF16, tag="bt", name="btile")
                    nc.sync.dma_start(
                        out=btile,
                        in_=bt[512 * g:512 * (g + 1), 512 * b:512 * (b + 1)]
                        .rearrange("(j p) q -> p j q", p=128))
                    pts = []
                    for h in range(HG):
                        pex = spool.tile([128, 4, 512], BF16, tag="pex", name="pex")
                        for jj in range(2):
                            ps = ps_main.tile([128, 2, 512], F32, tag="ps", name="ps")
                            for j2 in range(2):
                                c = 4 * g + 2 * jj + j2
                                nc.tensor.matmul(
                                    ps[:, j2, :],
                                    kT[h][:, 128 * c:128 * (c + 1)],
                                    qT[h][:, 512 * b:512 * (b + 1)],
                                    start=True, stop=True)
                            nc.scalar.activation(
                                pex[:, 2 * jj:2 * jj + 2, :], ps, EXP, scale=SCALE)
                        pt = ppool.tile([128, 4, 512], BF16, tag="pt", name="pt")
                        nc.vector.tensor_mul(pt, pex, btile)
                        pts.append(pt)
                    return pts

                def emit_av_group(b, g, pts):
                    if g == 0:
                        oaug[b] = [ps_oaug.tile([65, 512], F32, tag="oaug",
                                                name="oaug") for _ in range(HG)]
                    for h in range(HG):
                        for j in range(4):
                            c = 4 * g + j
                            nc.tensor.matmul(
                                oaug[b][h], vt[:, c, h, :], pts[h][:, j, :],
                                start=(c == 0), stop=(c == KC - 1))

                def emit_recips(b):
                    """1/den per head as exp(-ln(den)) on ACT: Ln and Exp
                    share one activation table (natural_log_exp_and_others),
                    so this is two fast table ops instead of the slow DVE
                    reciprocal (~8 cycles/elem)."""
                    recs = []
                    for h in range(HG):
                        lnd = npool.tile([1, 512], F32, tag="lnd", bufs=3,
                                         name="lnd")
                        nc.scalar.activation(lnd, oaug[b][h][64:65, :], LN)
                        rec = npool.tile([1, 512], mybir.dt.float32r, tag="rec",
                                         bufs=3, name="rec")
                        with nc.allow_low_precision(reason="f32r is f32 bits"):
                            nc.scalar.activation(rec, lnd, EXP, scale=-1.0)
                        recs.append(rec)
                    return recs

                def emit_bcast_one(h, recs):
                    """PE-broadcast 1/den across 64 partitions, evict via DVE."""
                    rbp = ps_rb.tile([64, 512], F32, tag="rbp", bufs=1, name="rbp")
                    with nc.allow_low_precision(reason="f32r is f32 bits"):
                        nc.tensor.matmul(rbp, ones_sb[0:1, :],
                                         recs[h][0:1, :],
                                         start=True, stop=True)
                    recb = npool.tile([64, 512], F32, tag="recb", bufs=2,
                                      name="recb")
                    nc.vector.tensor_copy(recb, rbp)
                    return recb

                def emit_norm_mul(b, recbs):
                    otn = []
                    for h in range(HG):
                        on = npool.tile([64, 512], BF16, tag="otn", bufs=4, name="on")
                        with nc.allow_low_precision(reason="bf16 attn output"):
                            nc.vector.tensor_mul(on, oaug[b][h][0:64, :], recbs[h])
                        otn.append(on)
                    return otn

                def emit_outproj(b, otn):
                    for t in range(4):
                        po = ps_main.tile([128, 768], F32, tag="ps", name="po")
                        for h in range(HG):
                            for e0, e1 in ((0, 512), (512, 768)):
                                nc.tensor.matmul(
                                    po[:, e0:e1],
                                    otn[h][:, 128 * t:128 * (t + 1)],
                                    wo_sb[:, h, e0:e1],
                                    start=(h == 0), stop=(h == HG - 1))
                        osb = opool.tile([128, 768], F32, tag="os", name="osb")
                        nc.vector.tensor_copy(osb, po)
                        qrow = (b * 4 + t) * 128
                        nc.sync.dma_start(out=outp[qrow:qrow + 128, :], in_=osb)

                # software-pipelined attention: block b's normalize/out-proj
                # interleaves with block b+1's first score group so the PE
                # stream stays dense across block boundaries. The rbp
                # broadcast chain ping-pongs PE<->DVE through a single PSUM
                # bank, so b+1's score matmuls are emitted between rbp0 and
                # rbp1 to hide each hop's latency.
                pts_carry = None
                for b in range(QB):
                    if pts_carry is not None:
                        emit_av_group(b, 0, pts_carry)
                    for g in (range(1, 8) if pts_carry is not None else range(8)):
                        pts = emit_score_group(b, g)
                        emit_av_group(b, g, pts)
                    recs = emit_recips(b)
                    recbs = [emit_bcast_one(0, recs)]
                    pts_carry = (emit_score_group(b + 1, 0)
                                 if b + 1 < QB else None)
                    recbs.append(emit_bcast_one(1, recs))
                    recbs.append(emit_bcast_one(2, recs))
                    otn = emit_norm_mul(b, recbs)
                    emit_outproj(b, otn)
                    # next block's AV for g=0 is emitted at the loop top

    _legalize_waits(nc, mybir)
    return nc


def _host_prep(inputs):
    import ml_dtypes

    BF = ml_dtypes.bfloat16
    x = np.asarray(inputs["x"], dtype=np.float32)[0]          # [N, E]
    sm = np.asarray(inputs["similarity_matrix"]).astype(np.int64)  # [N, 5, 2]
    Wq = np.asarray(inputs["Wq"], dtype=np.float32)
    bq = np.asarray(inputs["bq"], dtype=np.float32)
    Wk = np.asarray(inputs["Wk"], dtype=np.float32)
    Wv = np.asarray(inputs["Wv"], dtype=np.float32)
    Wo = np.asarray(inputs["Wo"], dtype=np.float32)

    has_bq = bool(np.any(bq != 0.0))
    xT = np.ascontiguousarray(x.T).astype(BF)                 # [E, N]

    # dense bias multiplier: exp(count) at each (query, key), transposed to
    # [keys, queries] to match the S^T score layout
    idx = sm.reshape(N, -1)
    vals = np.where(idx < N, 1.0, 0.0).astype(np.float32)
    safe = np.minimum(idx, N - 1)
    Bm = np.zeros((N, N), dtype=np.float32)
    np.add.at(Bm, (np.repeat(np.arange(N), idx.shape[1]), safe.reshape(-1)),
              vals.reshape(-1))
    MT = np.exp(Bm.T).astype(BF)                              # [keys, queries]

    in_maps = []
    for core in range(8):
        g, s = core // NS, core % NS
        gcols = slice(g * DG, (g + 1) * DG)
        wq_np = np.ascontiguousarray(Wq[gcols, :].T).astype(BF)  # [E, 192]
        wkv_np = np.zeros((E, HG * 128), dtype=BF)
        wb_np = np.zeros((E, 96), dtype=BF)
        for h in range(HG):
            hc = slice((g * HG + h) * D, (g * HG + h + 1) * D)
            wkv_np[:, 128 * h:128 * h + 64] = Wk[hc, :].T.astype(BF)
            wkv_np[:, 128 * h + 64:128 * h + 128] = Wv[hc, :].T.astype(BF)
            wb_np[:, 32 * h] = (Wk[hc, :].T @ bq[hc]).astype(BF)  # beta weights
        wo_np = np.ascontiguousarray(Wo[:, gcols].T).astype(BF)  # [192, E]
        in_maps.append({
            "xT": xT,
            "xTq": np.ascontiguousarray(xT[:, s * Q:(s + 1) * Q]),
            "wkv": wkv_np, "wq": wq_np, "wb": wb_np, "wo": wo_np,
            "bt": np.ascontiguousarray(MT[:, s * Q:(s + 1) * Q]),
            "ones_a": np.ones((128, KC * HG), dtype=BF),
            "ones_f": np.ones((1, 64), dtype=np.float32),
            "ones_b": np.ones((3, Q), dtype=BF),
            "ident": np.eye(64, dtype=BF),
        })
    return in_maps, has_bq


def kernel(**inputs):
    from concourse.bass_utils import run_bass_kernel_spmd

    in_maps, has_bq = _host_prep(inputs)
    key = ("prog", has_bq)
    if key not in _prog_cache:
        _prog_cache[key] = _build_program(has_bq)
    nc = _prog_cache[key]

    res = run_bass_kernel_spmd(nc, in_maps, list(range(8)))

    bv = np.asarray(inputs["bv"], dtype=np.float32)
    bo = np.asarray(inputs["bo"], dtype=np.float32)
    Wo = np.asarray(inputs["Wo"], dtype=np.float32)

    full = np.zeros((N, E), dtype=np.float32)
    for core in range(8):
        s = core % NS
        full[s * Q:(s + 1) * Q, :] += res.results[core]["outp"]
    full += (bv @ Wo.T + bo)[None, :]
    return full.reshape(B, N, E)
